# revision 5
# baseline (speedup 1.0000x reference)
"""Trainium2 Bass kernel for batched ADMM sparse-coding iterations (nn_DAD_84507776516366).

Math (from the reference):
    d        = diag(inv(L)) * diag(inv(U))  with  P L U = a^T a + rho phi^T phi
    x0       = y @ a
    zmu_0 = 0; u_0 = 0
    for t in 1..10:
        x_hat_t = d * (x0 + zmu_{t-1} @ phi)          (rho = 1)
        fxu_t   = x_hat_t @ phi^T + u_{t-1}
        u_t     = clamp(fxu_t, -lam, lam)             (== u + fxu - shrink(fxu))
        zmu_t   = fxu_t - 2 u_t                       (== z_t - u_t)
    return clip(x_hat_10, min(x), max(x))

Sharding: data-parallel over batch, 256 rows per core on 8 cores. a/phi/d
replicated. The small LU is computed host-side once (the "replicated
diagonals" of the sharding hint).

On-device layout: everything is batch-in-free-dim (transposed). State per
core: u [S,B], zmu [S,B] (f32r), x_hat [A,B] (f32r), d*x0 [A,B]. phi is
streamed from HBM twice per iteration in two pre-tiled layouts ([S,A] tiles
for x_hat matmuls, [A,S] tiles for fxu matmuls); matmuls run in float32r
(full PE rate).
"""
import numpy as np

RHO = 1.0
LAMDA = 0.1
ADMM_ITERS = 10
N_CORES = 8

B_FULL, M_DIM, A_DIM, S_DIM = 2048, 512, 2048, 6144
B_LOC = B_FULL // N_CORES


def build_program(B, AJ, SM, MK, iters, n_cores=N_CORES, shrink=LAMDA / RHO,
                  use_f32r=False):
    """Build the bass program. Dimensions in units of 128-partitions:
    A = AJ*128, S = SM*128, M = MK*128; B = per-core batch (free dim)."""
    from contextlib import ExitStack
    import concourse.bacc as bacc
    import concourse.tile as tile
    import concourse.mybir as mybir

    F32 = mybir.dt.float32
    F32R = mybir.dt.float32r if use_f32r else F32
    OP = mybir.AluOpType

    KH = 16 if SM % 16 == 0 else SM  # k-tiles per streamed phi tile (mm1)
    assert SM % KH == 0
    H = SM // KH

    nc = bacc.Bacc("TRN2", target_bir_lowering=False, debug=False,
                   enable_asserts=False, num_devices=n_cores)

    yT = nc.dram_tensor("yT", [128, MK, B], F32R, kind="ExternalInput").ap()
    a_sb = nc.dram_tensor("a_sb", [AJ, 128, MK, 128], F32R, kind="ExternalInput").ap()
    phi_sb = nc.dram_tensor("phi_sb", [AJ, 128, SM, 128], F32R, kind="ExternalInput").ap()
    phiT_sb = nc.dram_tensor("phiT_sb", [SM, 128, AJ, 128], F32R, kind="ExternalInput").ap()
    d_sb = nc.dram_tensor("d_sb", [128, AJ], F32, kind="ExternalInput").ap()
    mm_sb = nc.dram_tensor("mm_sb", [128, 2], F32, kind="ExternalInput").ap()
    xout = nc.dram_tensor("xout", [128, AJ, B], F32, kind="ExternalOutput").ap()

    with tile.TileContext(nc) as tc:
        with ExitStack() as ctx:
            const = ctx.enter_context(tc.tile_pool(name="const", bufs=1))
            state = ctx.enter_context(tc.tile_pool(name="state", bufs=1))
            stream = ctx.enter_context(tc.tile_pool(name="stream", bufs=6))
            tmp = ctx.enter_context(tc.tile_pool(name="tmp", bufs=4))
            ps1 = ctx.enter_context(tc.tile_pool(name="ps1", bufs=4, space="PSUM"))
            ps2 = ctx.enter_context(tc.tile_pool(name="ps2", bufs=3, space="PSUM"))

            d_t = const.tile([128, AJ], F32)
            nc.sync.dma_start(out=d_t[:], in_=d_sb[:])
            mm_t = const.tile([128, 2], F32)
            nc.sync.dma_start(out=mm_t[:], in_=mm_sb[:])
            yT_t = const.tile([128, MK, B], F32R)
            nc.sync.dma_start(out=yT_t[:], in_=yT[:])

            x0d = state.tile([128, AJ, B], F32)
            u_t = state.tile([128, SM, B], F32)
            zmu = state.tile([128, SM, B], F32R)
            xh = state.tile([128, AJ, B], F32R)

            # ---- prologue: x0d = d * (y @ a)^T ; xh = x0d (= x_hat_1) ----
            for j in range(AJ):
                at = stream.tile([128, KH, 128], F32R, name="stream", tag="stream")
                nc.sync.dma_start(out=at[:, :MK, :], in_=a_sb[j])
                p = ps1.tile([128, B], F32, name="p1", tag="p1")
                for k in range(MK):
                    nc.tensor.matmul(p[:], at[:, k, :], yT_t[:, k, :],
                                     start=(k == 0), stop=(k == MK - 1))
                nc.vector.tensor_scalar(x0d[:, j, :], p[:], d_t[:, j:j + 1], None,
                                        OP.mult)
                nc.vector.tensor_copy(xh[:, j, :], x0d[:, j, :])

            def mm2_phase(first):
                # fxu^T[S,B] = phi @ xh ; then u/zmu updates
                for m in range(SM):
                    pt = stream.tile([128, KH, 128], F32R, name="stream", tag="stream")
                    nc.sync.dma_start(out=pt[:, :AJ, :], in_=phiT_sb[m])
                    p = ps2.tile([128, B], F32, name="p2", tag="p2")
                    for k in range(AJ):
                        nc.tensor.matmul(p[:], pt[:, k, :], xh[:, k, :],
                                         start=(k == 0), stop=(k == AJ - 1))
                    if first:
                        # c = clamp(mm2); u = 0 + c; zmu = mm2 - 2c
                        nc.vector.tensor_scalar(u_t[:, m, :], p[:], shrink, -shrink,
                                                OP.min, OP.max)
                        nc.vector.scalar_tensor_tensor(zmu[:, m, :], u_t[:, m, :],
                                                       -2.0, p[:], OP.mult, OP.add)
                    else:
                        # fxu = mm2 + u; c = clamp(fxu); u += c; zmu = mm2 - 2c
                        f = tmp.tile([128, B], F32, name="fxu", tag="fxu")
                        nc.vector.tensor_add(f[:], p[:], u_t[:, m, :])
                        c = tmp.tile([128, B], F32, name="fxu", tag="fxu")
                        nc.vector.tensor_scalar(c[:], f[:], shrink, -shrink,
                                                OP.min, OP.max)
                        nc.vector.tensor_add(u_t[:, m, :], u_t[:, m, :], c[:])
                        nc.vector.scalar_tensor_tensor(zmu[:, m, :], c[:],
                                                       -2.0, p[:], OP.mult, OP.add)

            def mm1_phase(last):
                # x_hat^T[A,B] = d * (x0^T + phi^T @ zmu)
                for j in range(AJ):
                    p = ps1.tile([128, B], F32, name="p1", tag="p1")
                    for h in range(H):
                        ph = stream.tile([128, KH, 128], F32R, name="stream",
                                         tag="stream")
                        nc.sync.dma_start(out=ph[:],
                                          in_=phi_sb[j, :, h * KH:(h + 1) * KH, :])
                        for kk in range(KH):
                            k = h * KH + kk
                            nc.tensor.matmul(p[:], ph[:, kk, :], zmu[:, k, :],
                                             start=(k == 0), stop=(k == SM - 1))
                    if last:
                        t = tmp.tile([128, B], F32, name="fxu", tag="fxu")
                        nc.vector.scalar_tensor_tensor(t[:], p[:], d_t[:, j:j + 1],
                                                       x0d[:, j, :], OP.mult, OP.add)
                        xo = tmp.tile([128, B], F32, name="fxu", tag="fxu")
                        nc.vector.tensor_scalar(xo[:], t[:], mm_t[:, 0:1],
                                                mm_t[:, 1:2], OP.max, OP.min)
                        nc.sync.dma_start(out=xout[:, j, :], in_=xo[:])
                    else:
                        nc.vector.scalar_tensor_tensor(xh[:, j, :], p[:],
                                                       d_t[:, j:j + 1], x0d[:, j, :],
                                                       OP.mult, OP.add)

            # ---- iteration 1: mm2 only (zmu_0 = 0 makes mm1 trivial) ----
            mm2_phase(first=True)

            # ---- iterations 2 .. iters-1 ----
            if iters > 2:
                with tc.For_i(0, iters - 2, 1):
                    mm1_phase(last=False)
                    mm2_phase(first=False)

            # ---- iteration `iters`: x_hat only, clipped to [min x, max x] ----
            mm1_phase(last=True)

    nc.compile()
    return nc


def host_prepare(y, x, a, phi):
    """Host-side prep: LU-derived diagonals + pre-tiled / sharded arrays."""
    import scipy.linalg as sla

    y = np.asarray(y, dtype=np.float32)
    x = np.asarray(x, dtype=np.float32)
    a = np.asarray(a, dtype=np.float32)
    phi = np.asarray(phi, dtype=np.float32)

    B, M = y.shape
    A = a.shape[1]
    S = phi.shape[0]
    AJ, SM, MK = A // 128, S // 128, M // 128
    Bl = B // N_CORES

    m = a.T @ a + np.float32(RHO) * (phi.T @ phi)
    _, L, U = sla.lu(m)
    dL = np.ascontiguousarray(np.diag(np.linalg.inv(L))).astype(np.float32)
    dU = (np.float32(1.0) / np.diag(U)).astype(np.float32)
    d = dL * dU

    d_sb = np.ascontiguousarray(np.broadcast_to(d.reshape(AJ, 128).T, (128, AJ)))
    mm_sb = np.empty((128, 2), np.float32)
    mm_sb[:, 0] = x.min()
    mm_sb[:, 1] = x.max()

    phi_r = phi.reshape(SM, 128, AJ, 128)
    phi_sb = np.ascontiguousarray(phi_r.transpose(2, 1, 0, 3))   # [AJ,128,SM,128]
    phiT_sb = np.ascontiguousarray(phi_r.transpose(0, 3, 2, 1))  # [SM,128,AJ,128]
    a_sb = np.ascontiguousarray(
        a.reshape(MK, 128, AJ, 128).transpose(2, 1, 0, 3))       # [AJ,128,MK,128]

    in_maps = []
    for c in range(N_CORES):
        yT = np.ascontiguousarray(
            y[c * Bl:(c + 1) * Bl].T.reshape(MK, 128, Bl).transpose(1, 0, 2))
        in_maps.append({
            "yT": yT, "a_sb": a_sb, "phi_sb": phi_sb, "phiT_sb": phiT_sb,
            "d_sb": d_sb, "mm_sb": mm_sb,
        })
    return in_maps


_CACHE = {}


def kernel(y, x, a, phi):
    from concourse.bass_utils import run_bass_kernel_spmd

    y = np.asarray(y, dtype=np.float32)
    B, M = np.asarray(y).shape
    A = np.asarray(a).shape[1]
    S = np.asarray(phi).shape[0]

    key = (B, M, A, S)
    warm = key in _CACHE
    if not warm:
        _CACHE[key] = build_program(B // N_CORES, A // 128, S // 128, M // 128,
                                    ADMM_ITERS)
    nc = _CACHE[key]

    in_maps = host_prepare(y, x, a, phi)
    if not warm:
        # warmup execution: the very first post-compile run has produced
        # corrupted outputs once; discard it and use a fresh execution.
        run_bass_kernel_spmd(nc, in_maps, core_ids=list(range(N_CORES)))
    res = run_bass_kernel_spmd(nc, in_maps, core_ids=list(range(N_CORES)))

    Bl = B // N_CORES
    out = np.empty((B, A), np.float32)
    for c in range(N_CORES):
        r = res.results[c]["xout"]                # [128, AJ, Bl]
        out[c * Bl:(c + 1) * Bl] = r.transpose(2, 1, 0).reshape(Bl, A)
    return out


# revision 8
# speedup vs baseline: 1.5050x; 1.5050x over previous
"""Trainium2 Bass kernel for batched ADMM sparse-coding iterations (nn_DAD_84507776516366).

Math (from the reference):
    d        = diag(inv(L)) * diag(inv(U))  with  P L U = a^T a + rho phi^T phi
    x0       = y @ a
    zmu_0 = 0; u_0 = 0
    for t in 1..10:
        x_hat_t = d * (x0 + zmu_{t-1} @ phi)          (rho = 1)
        mm2_t   = x_hat_t @ phi^T ; fxu_t = mm2_t + u_{t-1}
        c_t     = clamp(fxu_t, -lam, lam)
        u_t     = u_{t-1} + c_t
        zmu_t   = mm2_t - 2 c_t                       (== z_t - u_t)
    return clip(x_hat_10, min(x), max(x))

Sharding: data-parallel over batch, 256 rows per core on 8 cores. a/phi/d
replicated; the small LU runs host-side once (the "replicated diagonals"
of the sharding hint).

On-device layout is batch-in-free-dim (transposed): state per core is
u [S,B] (f32), zmu [S,B] and x_hat [A,B] (each as a bf16 hi/lo pair), and
d*x0 [A,B] (f32). phi is streamed from HBM twice per iteration in two
pre-tiled layouts, as bf16 hi/lo pairs (same bytes as f32).

Matmuls use an error-compensated bf16 split: p @ q ~= ph@qh + ph@ql + pl@qh
with ph=bf16(p), pl=bf16(p-ph). Each bf16 matmul runs at 1 cycle/row vs
fp32's 4, so this is 3/4 the PE time of native fp32 at ~1e-5 relative
error per product — end-to-end ~4e-4 vs the fp32 reference.
"""
import numpy as np

RHO = 1.0
LAMDA = 0.1
ADMM_ITERS = 10
N_CORES = 8

B_FULL, M_DIM, A_DIM, S_DIM = 2048, 512, 2048, 6144
B_LOC = B_FULL // N_CORES


def build_program(B, AJ, SM, MK, iters, n_cores=N_CORES, shrink=LAMDA / RHO):
    """Build the bass program. Dimensions in units of 128-partitions:
    A = AJ*128, S = SM*128, M = MK*128; B = per-core batch (free dim)."""
    from contextlib import ExitStack
    import concourse.bacc as bacc
    import concourse.tile as tile
    import concourse.mybir as mybir

    F32 = mybir.dt.float32
    BF16 = mybir.dt.bfloat16
    OP = mybir.AluOpType

    KH = 16 if SM % 16 == 0 else SM  # k-tiles per streamed phi tile (mm1)
    assert SM % KH == 0
    H = SM // KH

    nc = bacc.Bacc("TRN2", target_bir_lowering=False, debug=False,
                   enable_asserts=False, num_devices=n_cores)

    yT = nc.dram_tensor("yT", [128, MK, B], F32, kind="ExternalInput").ap()
    a_sb = nc.dram_tensor("a_sb", [AJ, 128, MK, 128], F32, kind="ExternalInput").ap()
    phi_h = nc.dram_tensor("phi_h", [AJ, 128, SM, 128], BF16, kind="ExternalInput").ap()
    phi_l = nc.dram_tensor("phi_l", [AJ, 128, SM, 128], BF16, kind="ExternalInput").ap()
    phiT_h = nc.dram_tensor("phiT_h", [SM, 128, AJ, 128], BF16, kind="ExternalInput").ap()
    phiT_l = nc.dram_tensor("phiT_l", [SM, 128, AJ, 128], BF16, kind="ExternalInput").ap()
    d_sb = nc.dram_tensor("d_sb", [128, AJ], F32, kind="ExternalInput").ap()
    mm_sb = nc.dram_tensor("mm_sb", [128, 2], F32, kind="ExternalInput").ap()
    xout = nc.dram_tensor("xout", [128, AJ, B], F32, kind="ExternalOutput").ap()

    with tile.TileContext(nc) as tc:
        with ExitStack() as ctx:
            const = ctx.enter_context(tc.tile_pool(name="const", bufs=1))
            state = ctx.enter_context(tc.tile_pool(name="state", bufs=1))
            stream = ctx.enter_context(tc.tile_pool(name="stream", bufs=8))
            tmp = ctx.enter_context(tc.tile_pool(name="tmp", bufs=4))
            ps1 = ctx.enter_context(tc.tile_pool(name="ps1", bufs=4, space="PSUM"))
            ps2 = ctx.enter_context(tc.tile_pool(name="ps2", bufs=3, space="PSUM"))

            d_t = const.tile([128, AJ], F32)
            nc.sync.dma_start(out=d_t[:], in_=d_sb[:])
            mm_t = const.tile([128, 2], F32)
            nc.sync.dma_start(out=mm_t[:], in_=mm_sb[:])
            yT_t = const.tile([128, MK, B], F32)
            nc.sync.dma_start(out=yT_t[:], in_=yT[:])

            x0d = state.tile([128, AJ, B], F32)
            u_t = state.tile([128, SM, B], F32)
            zmu_h = state.tile([128, SM, B], BF16)
            zmu_l = state.tile([128, SM, B], BF16)
            xh_h = state.tile([128, AJ, B], BF16)
            xh_l = state.tile([128, AJ, B], BF16)

            def write_pair(hi, lo, src32):
                # hi = bf16(v); lo = bf16(v - hi)
                nc.vector.tensor_copy(hi, src32)
                nc.vector.tensor_sub(lo, src32, hi)

            # ---- prologue: x0d = d * (y @ a)^T ; xh pair = split(x0d) ----
            for j in range(AJ):
                at = stream.tile([128, MK, 128], F32, name="stream", tag="stream")
                nc.sync.dma_start(out=at[:], in_=a_sb[j])
                p = ps1.tile([128, B], F32, name="p1", tag="p1")
                for k in range(MK):
                    nc.tensor.matmul(p[:], at[:, k, :], yT_t[:, k, :],
                                     start=(k == 0), stop=(k == MK - 1))
                nc.vector.tensor_scalar(x0d[:, j, :], p[:], d_t[:, j:j + 1], None,
                                        OP.mult)
                write_pair(xh_h[:, j, :], xh_l[:, j, :], x0d[:, j, :])

            def mm_split(p, w_h, w_l, m_h, m_l, first, last):
                nc.tensor.matmul(p, w_h, m_h, start=first, stop=False)
                nc.tensor.matmul(p, w_h, m_l, start=False, stop=False)
                nc.tensor.matmul(p, w_l, m_h, start=False, stop=last)

            def mm2_phase(first):
                # mm2^T[S,B] = phi @ xh ; then u/zmu updates
                for m in range(SM):
                    pth = stream.tile([128, KH, 128], BF16, name="stream", tag="stream")
                    nc.sync.dma_start(out=pth[:, :AJ, :], in_=phiT_h[m])
                    ptl = stream.tile([128, KH, 128], BF16, name="stream", tag="stream")
                    nc.sync.dma_start(out=ptl[:, :AJ, :], in_=phiT_l[m])
                    p = ps2.tile([128, B], F32, name="p2", tag="p2")
                    for k in range(AJ):
                        mm_split(p[:], pth[:, k, :], ptl[:, k, :],
                                 xh_h[:, k, :], xh_l[:, k, :],
                                 k == 0, k == AJ - 1)
                    if first:
                        # c = clamp(mm2); u = c; zmu = mm2 - 2c
                        nc.vector.tensor_scalar(u_t[:, m, :], p[:], shrink, -shrink,
                                                OP.min, OP.max)
                        z32 = tmp.tile([128, B], F32, name="fxu", tag="fxu")
                        nc.vector.scalar_tensor_tensor(z32[:], u_t[:, m, :],
                                                       -2.0, p[:], OP.mult, OP.add)
                    else:
                        # fxu = mm2 + u; c = clamp(fxu); u += c; zmu = mm2 - 2c
                        f = tmp.tile([128, B], F32, name="fxu", tag="fxu")
                        nc.vector.tensor_add(f[:], p[:], u_t[:, m, :])
                        c = tmp.tile([128, B], F32, name="fxu", tag="fxu")
                        nc.vector.tensor_scalar(c[:], f[:], shrink, -shrink,
                                                OP.min, OP.max)
                        nc.vector.tensor_add(u_t[:, m, :], u_t[:, m, :], c[:])
                        z32 = tmp.tile([128, B], F32, name="fxu", tag="fxu")
                        nc.vector.scalar_tensor_tensor(z32[:], c[:],
                                                       -2.0, p[:], OP.mult, OP.add)
                    write_pair(zmu_h[:, m, :], zmu_l[:, m, :], z32[:])

            def mm1_phase(last):
                # x_hat^T[A,B] = d * (x0^T + phi^T @ zmu)
                for j in range(AJ):
                    p = ps1.tile([128, B], F32, name="p1", tag="p1")
                    for h in range(H):
                        ph_h = stream.tile([128, KH, 128], BF16, name="stream",
                                           tag="stream")
                        nc.sync.dma_start(out=ph_h[:],
                                          in_=phi_h[j, :, h * KH:(h + 1) * KH, :])
                        ph_l = stream.tile([128, KH, 128], BF16, name="stream",
                                           tag="stream")
                        nc.sync.dma_start(out=ph_l[:],
                                          in_=phi_l[j, :, h * KH:(h + 1) * KH, :])
                        for kk in range(KH):
                            k = h * KH + kk
                            mm_split(p[:], ph_h[:, kk, :], ph_l[:, kk, :],
                                     zmu_h[:, k, :], zmu_l[:, k, :],
                                     k == 0, k == SM - 1)
                    if last:
                        t = tmp.tile([128, B], F32, name="fxu", tag="fxu")
                        nc.vector.scalar_tensor_tensor(t[:], p[:], d_t[:, j:j + 1],
                                                       x0d[:, j, :], OP.mult, OP.add)
                        xo = tmp.tile([128, B], F32, name="fxu", tag="fxu")
                        nc.vector.tensor_scalar(xo[:], t[:], mm_t[:, 0:1],
                                                mm_t[:, 1:2], OP.max, OP.min)
                        nc.sync.dma_start(out=xout[:, j, :], in_=xo[:])
                    else:
                        x32 = tmp.tile([128, B], F32, name="fxu", tag="fxu")
                        nc.vector.scalar_tensor_tensor(x32[:], p[:],
                                                       d_t[:, j:j + 1], x0d[:, j, :],
                                                       OP.mult, OP.add)
                        write_pair(xh_h[:, j, :], xh_l[:, j, :], x32[:])

            # ---- iteration 1: mm2 only (zmu_0 = 0 makes mm1 trivial) ----
            mm2_phase(first=True)

            # ---- iterations 2 .. iters-1 ----
            if iters > 2:
                with tc.For_i(0, iters - 2, 1):
                    mm1_phase(last=False)
                    mm2_phase(first=False)

            # ---- iteration `iters`: x_hat only, clipped to [min x, max x] ----
            mm1_phase(last=True)

    nc.compile()
    return nc


def _split_pair(v):
    import ml_dtypes
    hi = v.astype(ml_dtypes.bfloat16)
    lo = (v - hi.astype(np.float32)).astype(ml_dtypes.bfloat16)
    return np.ascontiguousarray(hi), np.ascontiguousarray(lo)


def host_prepare(y, x, a, phi):
    """Host-side prep: LU-derived diagonals + pre-tiled / sharded arrays."""
    import scipy.linalg as sla

    y = np.asarray(y, dtype=np.float32)
    x = np.asarray(x, dtype=np.float32)
    a = np.asarray(a, dtype=np.float32)
    phi = np.asarray(phi, dtype=np.float32)

    B, M = y.shape
    A = a.shape[1]
    S = phi.shape[0]
    AJ, SM, MK = A // 128, S // 128, M // 128
    Bl = B // N_CORES

    m = a.T @ a + np.float32(RHO) * (phi.T @ phi)
    _, L, U = sla.lu(m)
    dL = np.ascontiguousarray(np.diag(np.linalg.inv(L))).astype(np.float32)
    dU = (np.float32(1.0) / np.diag(U)).astype(np.float32)
    d = dL * dU

    d_sb = np.ascontiguousarray(np.broadcast_to(d.reshape(AJ, 128).T, (128, AJ)))
    mm_sb = np.empty((128, 2), np.float32)
    mm_sb[:, 0] = x.min()
    mm_sb[:, 1] = x.max()

    phi_r = phi.reshape(SM, 128, AJ, 128)
    phi_sb = np.ascontiguousarray(phi_r.transpose(2, 1, 0, 3))   # [AJ,128,SM,128]
    phiT_sb = np.ascontiguousarray(phi_r.transpose(0, 3, 2, 1))  # [SM,128,AJ,128]
    phi_hc, phi_lc = _split_pair(phi_sb)
    phiT_hc, phiT_lc = _split_pair(phiT_sb)
    a_sb = np.ascontiguousarray(
        a.reshape(MK, 128, AJ, 128).transpose(2, 1, 0, 3))       # [AJ,128,MK,128]

    in_maps = []
    for c in range(N_CORES):
        yT = np.ascontiguousarray(
            y[c * Bl:(c + 1) * Bl].T.reshape(MK, 128, Bl).transpose(1, 0, 2))
        in_maps.append({
            "yT": yT, "a_sb": a_sb,
            "phi_h": phi_hc, "phi_l": phi_lc,
            "phiT_h": phiT_hc, "phiT_l": phiT_lc,
            "d_sb": d_sb, "mm_sb": mm_sb,
        })
    return in_maps


_CACHE = {}


def kernel(y, x, a, phi):
    from concourse.bass_utils import run_bass_kernel_spmd

    y = np.asarray(y, dtype=np.float32)
    B, M = np.asarray(y).shape
    A = np.asarray(a).shape[1]
    S = np.asarray(phi).shape[0]

    key = (B, M, A, S)
    warm = key in _CACHE
    if not warm:
        _CACHE[key] = build_program(B // N_CORES, A // 128, S // 128, M // 128,
                                    ADMM_ITERS)
    nc = _CACHE[key]

    in_maps = host_prepare(y, x, a, phi)
    if not warm:
        # warmup execution: the very first post-compile run has produced
        # corrupted outputs once; discard it and use a fresh execution.
        run_bass_kernel_spmd(nc, in_maps, core_ids=list(range(N_CORES)))
    res = run_bass_kernel_spmd(nc, in_maps, core_ids=list(range(N_CORES)))

    Bl = B // N_CORES
    out = np.empty((B, A), np.float32)
    for c in range(N_CORES):
        r = res.results[c]["xout"]                # [128, AJ, Bl]
        out[c * Bl:(c + 1) * Bl] = r.transpose(2, 1, 0).reshape(Bl, A)
    return out


# revision 9
# speedup vs baseline: 1.5486x; 1.0290x over previous
"""Trainium2 Bass kernel for batched ADMM sparse-coding iterations (nn_DAD_84507776516366).

Math (from the reference):
    d        = diag(inv(L)) * diag(inv(U))  with  P L U = a^T a + rho phi^T phi
    x0       = y @ a
    zmu_0 = 0; u_0 = 0
    for t in 1..10:
        x_hat_t = d * (x0 + zmu_{t-1} @ phi)          (rho = 1)
        mm2_t   = x_hat_t @ phi^T ; fxu_t = mm2_t + u_{t-1}
        c_t     = clamp(fxu_t, -lam, lam)
        u_t     = u_{t-1} + c_t
        zmu_t   = mm2_t - 2 c_t                       (== z_t - u_t)
    return clip(x_hat_10, min(x), max(x))

Sharding: data-parallel over batch, 256 rows per core on 8 cores. a/phi/d
replicated; the small LU runs host-side once (the "replicated diagonals"
of the sharding hint).

On-device layout is batch-in-free-dim (transposed): state per core is
u [S,B] (f32), zmu [S,B] and x_hat [A,B] (each as a bf16 hi/lo pair), and
d*x0 [A,B] (f32). phi is streamed from HBM twice per iteration in two
pre-tiled layouts, as bf16 hi/lo pairs (same bytes as f32).

Matmuls use an error-compensated bf16 split: p @ q ~= ph@qh + ph@ql + pl@qh
with ph=bf16(p), pl=bf16(p-ph). Each bf16 matmul runs at 1 cycle/row vs
fp32's 4, so this is 3/4 the PE time of native fp32 at ~1e-5 relative
error per product — end-to-end ~4e-4 vs the fp32 reference.
"""
import numpy as np

RHO = 1.0
LAMDA = 0.1
ADMM_ITERS = 10
N_CORES = 8

B_FULL, M_DIM, A_DIM, S_DIM = 2048, 512, 2048, 6144
B_LOC = B_FULL // N_CORES


def build_program(B, AJ, SM, MK, iters, n_cores=N_CORES, shrink=LAMDA / RHO):
    """Build the bass program. Dimensions in units of 128-partitions:
    A = AJ*128, S = SM*128, M = MK*128; B = per-core batch (free dim)."""
    from contextlib import ExitStack
    import concourse.bacc as bacc
    import concourse.tile as tile
    import concourse.mybir as mybir

    F32 = mybir.dt.float32
    BF16 = mybir.dt.bfloat16
    OP = mybir.AluOpType

    KH = 16 if SM % 16 == 0 else SM  # k-tiles per streamed phi tile (mm1)
    assert SM % KH == 0
    H = SM // KH

    nc = bacc.Bacc("TRN2", target_bir_lowering=False, debug=False,
                   enable_asserts=False, num_devices=n_cores)

    yT = nc.dram_tensor("yT", [128, MK, B], F32, kind="ExternalInput").ap()
    a_sb = nc.dram_tensor("a_sb", [AJ, 128, MK, 128], F32, kind="ExternalInput").ap()
    phi_h = nc.dram_tensor("phi_h", [AJ, 128, SM, 128], BF16, kind="ExternalInput").ap()
    phi_l = nc.dram_tensor("phi_l", [AJ, 128, SM, 128], BF16, kind="ExternalInput").ap()
    phiT_h = nc.dram_tensor("phiT_h", [SM, 128, AJ, 128], BF16, kind="ExternalInput").ap()
    phiT_l = nc.dram_tensor("phiT_l", [SM, 128, AJ, 128], BF16, kind="ExternalInput").ap()
    d_sb = nc.dram_tensor("d_sb", [128, AJ], F32, kind="ExternalInput").ap()
    mm_sb = nc.dram_tensor("mm_sb", [128, 2], F32, kind="ExternalInput").ap()
    xout = nc.dram_tensor("xout", [128, AJ, B], F32, kind="ExternalOutput").ap()

    with tile.TileContext(nc) as tc:
        with ExitStack() as ctx:
            const = ctx.enter_context(tc.tile_pool(name="const", bufs=1))
            state = ctx.enter_context(tc.tile_pool(name="state", bufs=1))
            stream = ctx.enter_context(tc.tile_pool(name="stream", bufs=8))
            tmp = ctx.enter_context(tc.tile_pool(name="tmp", bufs=4))
            ps1 = ctx.enter_context(tc.tile_pool(name="ps1", bufs=4, space="PSUM"))
            ps2 = ctx.enter_context(tc.tile_pool(name="ps2", bufs=3, space="PSUM"))

            d_t = const.tile([128, AJ], F32)
            nc.sync.dma_start(out=d_t[:], in_=d_sb[:])
            mm_t = const.tile([128, 2], F32)
            nc.sync.dma_start(out=mm_t[:], in_=mm_sb[:])
            yT_t = const.tile([128, MK, B], F32)
            nc.sync.dma_start(out=yT_t[:], in_=yT[:])

            x0d = state.tile([128, AJ, B], F32)
            u_t = state.tile([128, SM, B], F32)
            zmu_h = state.tile([128, SM, B], BF16)
            zmu_l = state.tile([128, SM, B], BF16)
            xh_h = state.tile([128, AJ, B], BF16)
            xh_l = state.tile([128, AJ, B], BF16)

            def write_pair(hi, lo, src32):
                # hi = bf16(v); lo = bf16(v - hi)
                nc.vector.tensor_copy(hi, src32)
                nc.vector.tensor_sub(lo, src32, hi)

            # ---- prologue: x0d = d * (y @ a)^T ; xh pair = split(x0d) ----
            for j in range(AJ):
                at = stream.tile([128, MK, 128], F32, name="stream", tag="stream")
                nc.sync.dma_start(out=at[:], in_=a_sb[j])
                p = ps1.tile([128, B], F32, name="p1", tag="p1")
                for k in range(MK):
                    nc.tensor.matmul(p[:], at[:, k, :], yT_t[:, k, :],
                                     start=(k == 0), stop=(k == MK - 1))
                nc.vector.tensor_scalar(x0d[:, j, :], p[:], d_t[:, j:j + 1], None,
                                        OP.mult)
                write_pair(xh_h[:, j, :], xh_l[:, j, :], x0d[:, j, :])

            def mm_split(p, w_h, w_l, m_h, m_l, first, last):
                nc.tensor.matmul(p, w_h, m_h, start=first, stop=False)
                nc.tensor.matmul(p, w_h, m_l, start=False, stop=False)
                nc.tensor.matmul(p, w_l, m_h, start=False, stop=last)

            def mm2_phase(first):
                # mm2^T[S,B] = phi @ xh ; then u/zmu updates
                for m in range(SM):
                    pth = stream.tile([128, KH, 128], BF16, name="stream", tag="stream")
                    nc.sync.dma_start(out=pth[:, :AJ, :], in_=phiT_h[m])
                    ptl = stream.tile([128, KH, 128], BF16, name="stream", tag="stream")
                    nc.sync.dma_start(out=ptl[:, :AJ, :], in_=phiT_l[m])
                    p = ps2.tile([128, B], F32, name="p2", tag="p2")
                    for k in range(AJ):
                        mm_split(p[:], pth[:, k, :], ptl[:, k, :],
                                 xh_h[:, k, :], xh_l[:, k, :],
                                 k == 0, k == AJ - 1)
                    if first:
                        # c = clamp(mm2); u = c; zmu = mm2 - 2c
                        nc.vector.tensor_scalar(u_t[:, m, :], p[:], shrink, -shrink,
                                                OP.min, OP.max)
                        z32 = tmp.tile([128, B], F32, name="fxu", tag="fxu")
                        nc.vector.scalar_tensor_tensor(z32[:], u_t[:, m, :],
                                                       -2.0, p[:], OP.mult, OP.add)
                    else:
                        # fxu = mm2 + u; c = clamp(fxu); u += c; zmu = mm2 - 2c
                        f = tmp.tile([128, B], F32, name="fxu", tag="fxu")
                        nc.vector.tensor_add(f[:], p[:], u_t[:, m, :])
                        c = tmp.tile([128, B], F32, name="fxu", tag="fxu")
                        nc.vector.tensor_scalar(c[:], f[:], shrink, -shrink,
                                                OP.min, OP.max)
                        nc.vector.tensor_add(u_t[:, m, :], u_t[:, m, :], c[:])
                        z32 = tmp.tile([128, B], F32, name="fxu", tag="fxu")
                        nc.vector.scalar_tensor_tensor(z32[:], c[:],
                                                       -2.0, p[:], OP.mult, OP.add)
                    write_pair(zmu_h[:, m, :], zmu_l[:, m, :], z32[:])

            def mm1_phase(last):
                # x_hat^T[A,B] = d * (x0^T + phi^T @ zmu)
                for j in range(AJ):
                    p = ps1.tile([128, B], F32, name="p1", tag="p1")
                    for h in range(H):
                        ph_h = stream.tile([128, KH, 128], BF16, name="stream",
                                           tag="stream")
                        nc.sync.dma_start(out=ph_h[:],
                                          in_=phi_h[j, :, h * KH:(h + 1) * KH, :])
                        ph_l = stream.tile([128, KH, 128], BF16, name="stream",
                                           tag="stream")
                        nc.sync.dma_start(out=ph_l[:],
                                          in_=phi_l[j, :, h * KH:(h + 1) * KH, :])
                        for kk in range(KH):
                            k = h * KH + kk
                            mm_split(p[:], ph_h[:, kk, :], ph_l[:, kk, :],
                                     zmu_h[:, k, :], zmu_l[:, k, :],
                                     k == 0, k == SM - 1)
                    if last:
                        t = tmp.tile([128, B], F32, name="fxu", tag="fxu")
                        nc.vector.scalar_tensor_tensor(t[:], p[:], d_t[:, j:j + 1],
                                                       x0d[:, j, :], OP.mult, OP.add)
                        xo = tmp.tile([128, B], F32, name="fxu", tag="fxu")
                        nc.vector.tensor_scalar(xo[:], t[:], mm_t[:, 0:1],
                                                mm_t[:, 1:2], OP.max, OP.min)
                        nc.sync.dma_start(out=xout[:, j, :], in_=xo[:])
                    else:
                        x32 = tmp.tile([128, B], F32, name="fxu", tag="fxu")
                        nc.vector.scalar_tensor_tensor(x32[:], p[:],
                                                       d_t[:, j:j + 1], x0d[:, j, :],
                                                       OP.mult, OP.add)
                        write_pair(xh_h[:, j, :], xh_l[:, j, :], x32[:])

            # ---- iteration 1: mm2 only (zmu_0 = 0 makes mm1 trivial) ----
            mm2_phase(first=True)

            # ---- iterations 2 .. iters-1 (unrolled: avoids the ~12us
            # all-engine back-edge barrier + HAM re-throttle per iteration) ----
            for _ in range(max(0, iters - 2)):
                mm1_phase(last=False)
                mm2_phase(first=False)

            # ---- iteration `iters`: x_hat only, clipped to [min x, max x] ----
            mm1_phase(last=True)

    nc.compile()
    return nc


def _split_pair(v):
    import ml_dtypes
    hi = v.astype(ml_dtypes.bfloat16)
    lo = (v - hi.astype(np.float32)).astype(ml_dtypes.bfloat16)
    return np.ascontiguousarray(hi), np.ascontiguousarray(lo)


def host_prepare(y, x, a, phi):
    """Host-side prep: LU-derived diagonals + pre-tiled / sharded arrays."""
    import scipy.linalg as sla

    y = np.asarray(y, dtype=np.float32)
    x = np.asarray(x, dtype=np.float32)
    a = np.asarray(a, dtype=np.float32)
    phi = np.asarray(phi, dtype=np.float32)

    B, M = y.shape
    A = a.shape[1]
    S = phi.shape[0]
    AJ, SM, MK = A // 128, S // 128, M // 128
    Bl = B // N_CORES

    m = a.T @ a + np.float32(RHO) * (phi.T @ phi)
    _, L, U = sla.lu(m)
    dL = np.ascontiguousarray(np.diag(np.linalg.inv(L))).astype(np.float32)
    dU = (np.float32(1.0) / np.diag(U)).astype(np.float32)
    d = dL * dU

    d_sb = np.ascontiguousarray(np.broadcast_to(d.reshape(AJ, 128).T, (128, AJ)))
    mm_sb = np.empty((128, 2), np.float32)
    mm_sb[:, 0] = x.min()
    mm_sb[:, 1] = x.max()

    phi_r = phi.reshape(SM, 128, AJ, 128)
    phi_sb = np.ascontiguousarray(phi_r.transpose(2, 1, 0, 3))   # [AJ,128,SM,128]
    phiT_sb = np.ascontiguousarray(phi_r.transpose(0, 3, 2, 1))  # [SM,128,AJ,128]
    phi_hc, phi_lc = _split_pair(phi_sb)
    phiT_hc, phiT_lc = _split_pair(phiT_sb)
    a_sb = np.ascontiguousarray(
        a.reshape(MK, 128, AJ, 128).transpose(2, 1, 0, 3))       # [AJ,128,MK,128]

    in_maps = []
    for c in range(N_CORES):
        yT = np.ascontiguousarray(
            y[c * Bl:(c + 1) * Bl].T.reshape(MK, 128, Bl).transpose(1, 0, 2))
        in_maps.append({
            "yT": yT, "a_sb": a_sb,
            "phi_h": phi_hc, "phi_l": phi_lc,
            "phiT_h": phiT_hc, "phiT_l": phiT_lc,
            "d_sb": d_sb, "mm_sb": mm_sb,
        })
    return in_maps


_CACHE = {}


def kernel(y, x, a, phi):
    from concourse.bass_utils import run_bass_kernel_spmd

    y = np.asarray(y, dtype=np.float32)
    B, M = np.asarray(y).shape
    A = np.asarray(a).shape[1]
    S = np.asarray(phi).shape[0]

    key = (B, M, A, S)
    warm = key in _CACHE
    if not warm:
        _CACHE[key] = build_program(B // N_CORES, A // 128, S // 128, M // 128,
                                    ADMM_ITERS)
    nc = _CACHE[key]

    in_maps = host_prepare(y, x, a, phi)
    if not warm:
        # warmup execution: the very first post-compile run has produced
        # corrupted outputs once; discard it and use a fresh execution.
        run_bass_kernel_spmd(nc, in_maps, core_ids=list(range(N_CORES)))
    res = run_bass_kernel_spmd(nc, in_maps, core_ids=list(range(N_CORES)))

    Bl = B // N_CORES
    out = np.empty((B, A), np.float32)
    for c in range(N_CORES):
        r = res.results[c]["xout"]                # [128, AJ, Bl]
        out[c * Bl:(c + 1) * Bl] = r.transpose(2, 1, 0).reshape(Bl, A)
    return out


# revision 16
# speedup vs baseline: 1.5608x; 1.0078x over previous
"""Trainium2 Bass kernel for batched ADMM sparse-coding iterations (nn_DAD_84507776516366).

Math (from the reference):
    d        = diag(inv(L)) * diag(inv(U))  with  P L U = a^T a + rho phi^T phi
    x0       = y @ a
    zmu_0 = 0; u_0 = 0
    for t in 1..10:
        x_hat_t = d * (x0 + zmu_{t-1} @ phi)          (rho = 1)
        mm2_t   = x_hat_t @ phi^T ; fxu_t = mm2_t + u_{t-1}
        c_t     = clamp(fxu_t, -lam, lam)
        u_t     = u_{t-1} + c_t
        zmu_t   = mm2_t - 2 c_t                       (== z_t - u_t)
    return clip(x_hat_10, min(x), max(x))

Sharding: data-parallel over batch, 256 rows per core on 8 cores. a/phi/d
replicated; the small LU runs host-side once (the "replicated diagonals"
of the sharding hint).

On-device layout is batch-in-free-dim (transposed): state per core is
u [S,B] (f32), zmu [S,B] and x_hat [A,B] (each as a bf16 hi/lo pair), and
d*x0 [A,B] (f32). phi is streamed from HBM twice per iteration in two
pre-tiled layouts, as bf16 hi/lo pairs (same bytes as f32).

Matmuls use an error-compensated bf16 split: p @ q ~= ph@qh + ph@ql + pl@qh
with ph=bf16(p), pl=bf16(p-ph). Each bf16 matmul runs at 1 cycle/row vs
fp32's 4, so this is 3/4 the PE time of native fp32 at ~1e-5 relative
error per product — end-to-end ~4e-4 vs the fp32 reference.
"""
import numpy as np

RHO = 1.0
LAMDA = 0.1
ADMM_ITERS = 10
N_CORES = 8

B_FULL, M_DIM, A_DIM, S_DIM = 2048, 512, 2048, 6144
B_LOC = B_FULL // N_CORES


def build_program(B, AJ, SM, MK, iters, n_cores=N_CORES, shrink=LAMDA / RHO):
    """Build the bass program. Dimensions in units of 128-partitions:
    A = AJ*128, S = SM*128, M = MK*128; B = per-core batch (free dim)."""
    from contextlib import ExitStack
    import concourse.bacc as bacc
    import concourse.tile as tile
    import concourse.mybir as mybir

    F32 = mybir.dt.float32
    BF16 = mybir.dt.bfloat16
    OP = mybir.AluOpType

    KH = 16 if SM % 16 == 0 else SM  # k-tiles per streamed phi tile (mm1)
    assert SM % KH == 0
    H = SM // KH

    nc = bacc.Bacc("TRN2", target_bir_lowering=False, debug=False,
                   enable_asserts=False, num_devices=n_cores)

    yT = nc.dram_tensor("yT", [128, MK, B], F32, kind="ExternalInput").ap()
    a_sb = nc.dram_tensor("a_sb", [AJ, 128, MK, 128], F32, kind="ExternalInput").ap()
    phi_h = nc.dram_tensor("phi_h", [AJ, 128, SM, 128], BF16, kind="ExternalInput").ap()
    phi_l = nc.dram_tensor("phi_l", [AJ, 128, SM, 128], BF16, kind="ExternalInput").ap()
    phiT_h = nc.dram_tensor("phiT_h", [SM, 128, AJ, 128], BF16, kind="ExternalInput").ap()
    phiT_l = nc.dram_tensor("phiT_l", [SM, 128, AJ, 128], BF16, kind="ExternalInput").ap()
    d_sb = nc.dram_tensor("d_sb", [128, AJ], F32, kind="ExternalInput").ap()
    mm_sb = nc.dram_tensor("mm_sb", [128, 2], F32, kind="ExternalInput").ap()
    xout = nc.dram_tensor("xout", [128, AJ, B], F32, kind="ExternalOutput").ap()

    with tile.TileContext(nc) as tc:
        with ExitStack() as ctx:
            const = ctx.enter_context(tc.tile_pool(name="const", bufs=1))
            state = ctx.enter_context(tc.tile_pool(name="state", bufs=1))
            stream = ctx.enter_context(tc.tile_pool(name="stream", bufs=8))
            tmp = ctx.enter_context(tc.tile_pool(name="tmp", bufs=6))
            ps1 = ctx.enter_context(tc.tile_pool(name="ps1", bufs=4, space="PSUM"))
            ps2 = ctx.enter_context(tc.tile_pool(name="ps2", bufs=3, space="PSUM"))

            d_t = const.tile([128, AJ], F32)
            nc.sync.dma_start(out=d_t[:], in_=d_sb[:])
            mm_t = const.tile([128, 2], F32)
            nc.sync.dma_start(out=mm_t[:], in_=mm_sb[:])
            yT_t = const.tile([128, MK, B], F32)
            nc.sync.dma_start(out=yT_t[:], in_=yT[:])

            x0d = state.tile([128, AJ, B], F32)
            u_t = state.tile([128, SM, B], F32)
            # hi/lo pairs stored adjacently so both can move through the PE
            # as one N=2B matmul against the shared hi weights
            zmu_p = state.tile([128, SM, 2, B], BF16)
            xh_p = state.tile([128, AJ, 2, B], BF16)

            def write_pair(hi, lo, src32):
                # hi = bf16(v); lo = bf16(v - hi)
                nc.vector.tensor_copy(hi, src32)
                nc.vector.tensor_sub(lo, src32, hi)

            # ---- prologue: x0d = d * (y @ a)^T ; xh pair = split(x0d) ----
            for j in range(AJ):
                at = stream.tile([128, MK, 128], F32, name="stream", tag="stream")
                nc.sync.dma_start(out=at[:], in_=a_sb[j])
                p = ps1.tile([128, 2, B], F32, name="p1", tag="p1")
                for k in range(MK):
                    nc.tensor.matmul(p[:, 0, :], at[:, k, :], yT_t[:, k, :],
                                     start=(k == 0), stop=(k == MK - 1))
                nc.vector.tensor_scalar(x0d[:, j, :], p[:, 0, :], d_t[:, j:j + 1],
                                        None, OP.mult)
                write_pair(xh_p[:, j, 0, :], xh_p[:, j, 1, :], x0d[:, j, :])

            def psum_pair_sum(dst, p):
                # DVE may read only ONE PSUM operand per instruction: stage
                # the lo half through SBUF, then add the hi half.
                t = tmp.tile([128, B], F32, name="fxu", tag="fxu")
                nc.vector.tensor_copy(t[:], p[:, 1, :])
                nc.vector.tensor_add(dst, p[:, 0, :], t[:])

            def mm_split(p, w_h, w_l, m_pair, first, last):
                # p[:,0:2B] += w_h.T @ [m_h | m_l]; p[:,0:B] += w_l.T @ m_h.
                # Halves are summed at eviction: out = p_hi_half + p_lo_half
                # (p[:,0,:] accumulates wh@mh + wl@mh, p[:,1,:] has wh@ml).
                nc.tensor.matmul(p[:, :, :], w_h, m_pair, start=first, stop=False)
                nc.tensor.matmul(p[:, 0, :], w_l, m_pair[:, 0, :],
                                 start=False, stop=last)

            def mm2_phase(first):
                # mm2^T[S,B] = phi @ xh ; then u/zmu updates
                for m in range(SM):
                    pth = stream.tile([128, KH, 128], BF16, name="stream", tag="stream")
                    nc.sync.dma_start(out=pth[:, :AJ, :], in_=phiT_h[m])
                    ptl = stream.tile([128, KH, 128], BF16, name="stream", tag="stream")
                    nc.sync.dma_start(out=ptl[:, :AJ, :], in_=phiT_l[m])
                    p = ps2.tile([128, 2, B], F32, name="p2", tag="p2")
                    for k in range(AJ):
                        mm_split(p[:], pth[:, k, :], ptl[:, k, :],
                                 xh_p[:, k, :, :], k == 0, k == AJ - 1)
                    s32 = tmp.tile([128, B], F32, name="fxu", tag="fxu")
                    psum_pair_sum(s32[:], p)  # mm2 = hi half + lo half
                    if first:
                        # c = clamp(mm2); u = c; zmu = mm2 - 2c
                        nc.vector.tensor_scalar(u_t[:, m, :], s32[:], shrink,
                                                -shrink, OP.min, OP.max)
                        z32 = tmp.tile([128, B], F32, name="fxu", tag="fxu")
                        nc.vector.scalar_tensor_tensor(z32[:], u_t[:, m, :],
                                                       -2.0, s32[:], OP.mult, OP.add)
                    else:
                        # fxu = mm2 + u; c = clamp(fxu); u += c; zmu = mm2 - 2c
                        f = tmp.tile([128, B], F32, name="fxu", tag="fxu")
                        nc.vector.tensor_add(f[:], s32[:], u_t[:, m, :])
                        c = tmp.tile([128, B], F32, name="fxu", tag="fxu")
                        nc.vector.tensor_scalar(c[:], f[:], shrink, -shrink,
                                                OP.min, OP.max)
                        nc.vector.tensor_add(u_t[:, m, :], u_t[:, m, :], c[:])
                        z32 = tmp.tile([128, B], F32, name="fxu", tag="fxu")
                        nc.vector.scalar_tensor_tensor(z32[:], c[:],
                                                       -2.0, s32[:], OP.mult, OP.add)
                    write_pair(zmu_p[:, m, 0, :], zmu_p[:, m, 1, :], z32[:])

            def mm1_phase(last):
                # x_hat^T[A,B] = d * (x0^T + phi^T @ zmu)
                for j in range(AJ):
                    p = ps1.tile([128, 2, B], F32, name="p1", tag="p1")
                    for h in range(H):
                        ph_h = stream.tile([128, KH, 128], BF16, name="stream",
                                           tag="stream")
                        nc.sync.dma_start(out=ph_h[:],
                                          in_=phi_h[j, :, h * KH:(h + 1) * KH, :])
                        ph_l = stream.tile([128, KH, 128], BF16, name="stream",
                                           tag="stream")
                        nc.sync.dma_start(out=ph_l[:],
                                          in_=phi_l[j, :, h * KH:(h + 1) * KH, :])
                        for kk in range(KH):
                            k = h * KH + kk
                            mm_split(p[:], ph_h[:, kk, :], ph_l[:, kk, :],
                                     zmu_p[:, k, :, :], k == 0, k == SM - 1)
                    s32 = tmp.tile([128, B], F32, name="fxu", tag="fxu")
                    psum_pair_sum(s32[:], p)
                    if last:
                        t = tmp.tile([128, B], F32, name="fxu", tag="fxu")
                        nc.vector.scalar_tensor_tensor(t[:], s32[:], d_t[:, j:j + 1],
                                                       x0d[:, j, :], OP.mult, OP.add)
                        xo = tmp.tile([128, B], F32, name="fxu", tag="fxu")
                        nc.vector.tensor_scalar(xo[:], t[:], mm_t[:, 0:1],
                                                mm_t[:, 1:2], OP.max, OP.min)
                        nc.sync.dma_start(out=xout[:, j, :], in_=xo[:])
                    else:
                        x32 = tmp.tile([128, B], F32, name="fxu", tag="fxu")
                        nc.vector.scalar_tensor_tensor(x32[:], s32[:],
                                                       d_t[:, j:j + 1], x0d[:, j, :],
                                                       OP.mult, OP.add)
                        write_pair(xh_p[:, j, 0, :], xh_p[:, j, 1, :], x32[:])

            # ---- iteration 1: mm2 only (zmu_0 = 0 makes mm1 trivial) ----
            mm2_phase(first=True)

            # ---- iterations 2 .. iters-1 (unrolled: avoids the ~12us
            # all-engine back-edge barrier + HAM re-throttle per iteration) ----
            for _ in range(max(0, iters - 2)):
                mm1_phase(last=False)
                mm2_phase(first=False)

            # ---- iteration `iters`: x_hat only, clipped to [min x, max x] ----
            mm1_phase(last=True)

    nc.compile()
    return nc


def _split_pair(v):
    import ml_dtypes
    hi = v.astype(ml_dtypes.bfloat16)
    lo = (v - hi.astype(np.float32)).astype(ml_dtypes.bfloat16)
    return np.ascontiguousarray(hi), np.ascontiguousarray(lo)


def host_prepare(y, x, a, phi):
    """Host-side prep: LU-derived diagonals + pre-tiled / sharded arrays."""
    import scipy.linalg as sla

    y = np.asarray(y, dtype=np.float32)
    x = np.asarray(x, dtype=np.float32)
    a = np.asarray(a, dtype=np.float32)
    phi = np.asarray(phi, dtype=np.float32)

    B, M = y.shape
    A = a.shape[1]
    S = phi.shape[0]
    AJ, SM, MK = A // 128, S // 128, M // 128
    Bl = B // N_CORES

    m = a.T @ a + np.float32(RHO) * (phi.T @ phi)
    _, L, U = sla.lu(m)
    dL = np.ascontiguousarray(np.diag(np.linalg.inv(L))).astype(np.float32)
    dU = (np.float32(1.0) / np.diag(U)).astype(np.float32)
    d = dL * dU

    d_sb = np.ascontiguousarray(np.broadcast_to(d.reshape(AJ, 128).T, (128, AJ)))
    mm_sb = np.empty((128, 2), np.float32)
    mm_sb[:, 0] = x.min()
    mm_sb[:, 1] = x.max()

    phi_r = phi.reshape(SM, 128, AJ, 128)
    phi_sb = np.ascontiguousarray(phi_r.transpose(2, 1, 0, 3))   # [AJ,128,SM,128]
    phiT_sb = np.ascontiguousarray(phi_r.transpose(0, 3, 2, 1))  # [SM,128,AJ,128]
    phi_hc, phi_lc = _split_pair(phi_sb)
    phiT_hc, phiT_lc = _split_pair(phiT_sb)
    a_sb = np.ascontiguousarray(
        a.reshape(MK, 128, AJ, 128).transpose(2, 1, 0, 3))       # [AJ,128,MK,128]

    in_maps = []
    for c in range(N_CORES):
        yT = np.ascontiguousarray(
            y[c * Bl:(c + 1) * Bl].T.reshape(MK, 128, Bl).transpose(1, 0, 2))
        in_maps.append({
            "yT": yT, "a_sb": a_sb,
            "phi_h": phi_hc, "phi_l": phi_lc,
            "phiT_h": phiT_hc, "phiT_l": phiT_lc,
            "d_sb": d_sb, "mm_sb": mm_sb,
        })
    return in_maps


_CACHE = {}


def kernel(y, x, a, phi):
    from concourse.bass_utils import run_bass_kernel_spmd

    y = np.asarray(y, dtype=np.float32)
    B, M = np.asarray(y).shape
    A = np.asarray(a).shape[1]
    S = np.asarray(phi).shape[0]

    key = (B, M, A, S)
    warm = key in _CACHE
    if not warm:
        _CACHE[key] = build_program(B // N_CORES, A // 128, S // 128, M // 128,
                                    ADMM_ITERS)
    nc = _CACHE[key]

    in_maps = host_prepare(y, x, a, phi)
    if not warm:
        # warmup execution: the very first post-compile run has produced
        # corrupted outputs once; discard it and use a fresh execution.
        run_bass_kernel_spmd(nc, in_maps, core_ids=list(range(N_CORES)))
    res = run_bass_kernel_spmd(nc, in_maps, core_ids=list(range(N_CORES)))

    Bl = B // N_CORES
    out = np.empty((B, A), np.float32)
    for c in range(N_CORES):
        r = res.results[c]["xout"]                # [128, AJ, Bl]
        out[c * Bl:(c + 1) * Bl] = r.transpose(2, 1, 0).reshape(Bl, A)
    return out
